# revision 1
# baseline (speedup 1.0000x reference)
"""Single-head attention (B=4, S=4096, E=2048, d=128) on 8 trn2 cores.

Sharding: core c handles (batch b = c//2, seq half h = c%2). Each core
computes the full K/V projection for its batch (redundantly within the
pair) and attention for its 2048-query half. The per-core input x is
rolled along S so the core's query rows are always rows 0:2048 — softmax
over keys is permutation-invariant, so rolling K/V order is harmless and
keeps the SPMD program identical across cores.

Per-core pipeline (all matmuls bf16, fp32 PSUM accumulation):
  xT tiles <- DMA-transpose of x (bf16)           [128e x 1024s] x 64
  qT/kT/vT <- W-stationary projection, bias folded into the ACT
              PSUM->SBUF evacuation (Identity activation, bias AP)
  v        <- PE transpose of vT chunks
  scoresT[k,q] = kT_chunk^T @ qT   (one matmul per 128-k chunk)
  expT     <- ACT Exp(scale * scoresT) straight from PSUM (no max
              subtraction: scores ~ N(0,1), exp is fp32-safe)
  out_T[d,q] += v_chunk^T @ expT   (PSUM accumulation over 32 chunks)
  sums[1,q] += ones^T @ expT       (softmax denominator, exact fp32)
Host: out = (out_T / sums).T per core, reassembled into [4,4096,128].
"""

import numpy as np
import ml_dtypes

import concourse.bass as bass
import concourse.tile as tile
from concourse import bacc, mybir
from concourse.bass_utils import run_bass_kernel_spmd
from concourse.masks import make_identity

N_CORES = 8
B, S, E, D = 4, 4096, 2048, 128
HALF = S // 2  # queries per core
QB = 512  # query block (PSUM bank width in fp32)
SCALE = 1.0 / float(np.sqrt(D))

BF16 = mybir.dt.bfloat16
F32 = mybir.dt.float32
AF = mybir.ActivationFunctionType

_CACHE = {}


def _build():
    nc = bacc.Bacc(trn_type="TRN2", target_bir_lowering=False, debug=False)

    x_d = nc.dram_tensor("x", [S, E], BF16, kind="ExternalInput").ap()
    w_d = nc.dram_tensor("w", [E, 3 * D], BF16, kind="ExternalInput").ap()
    bias_d = nc.dram_tensor("bias_cols", [D, 3], F32, kind="ExternalInput").ap()
    out_d = nc.dram_tensor("out_t", [D, HALF], F32, kind="ExternalOutput").ap()
    sums_d = nc.dram_tensor("sums", [1, HALF], F32, kind="ExternalOutput").ap()

    NE = E // 128  # 16 e-chunks
    NQ = 4  # s-quarters
    SQ = S // NQ  # 1024
    NKC = S // 128  # 32 k-chunks
    NQB = HALF // QB  # 4 query blocks

    with tile.TileContext(nc) as tc:
        with (
            tc.tile_pool(name="xt", bufs=40) as xt_pool,
            tc.tile_pool(name="wsb", bufs=1) as w_pool,
            tc.tile_pool(name="persist", bufs=1) as persist,
            tc.tile_pool(name="exp", bufs=6) as exp_pool,
            tc.tile_pool(name="osb", bufs=3) as out_pool,
            tc.tile_pool(name="ps_main", bufs=3, space="PSUM") as ps_main,
            tc.tile_pool(name="ps_acc", bufs=2, space="PSUM") as ps_acc,
            tc.tile_pool(name="ps_small", bufs=2, space="PSUM") as ps_small,
        ):
            # ---- constants / small inputs ----
            w_sb = w_pool.tile([128, NE * 3 * D], BF16, tag="w")
            for e in range(NE):
                nc.sync.dma_start(
                    w_sb[:, e * 3 * D : (e + 1) * 3 * D],
                    w_d[e * 128 : (e + 1) * 128, :],
                )
            bias_sb = persist.tile([D, 3], F32, tag="bias")
            nc.sync.dma_start(bias_sb[:], bias_d[:])
            ones_col = persist.tile([128, 1], BF16, tag="ones")
            nc.gpsimd.memset(ones_col[:], 1.0)
            ident = persist.tile([128, 128], BF16, tag="ident")
            make_identity(nc, ident[:])

            # ---- x loads (DMA transpose), per (quarter, e-chunk) ----
            xt = {}
            for sq in range(NQ):
                for e in range(NE):
                    t = xt_pool.tile([128, SQ], BF16, tag="xt")
                    nc.sync.dma_start_transpose(
                        t[:],
                        x_d[sq * SQ : (sq + 1) * SQ, e * 128 : (e + 1) * 128],
                    )
                    xt[(sq, e)] = t

            # ---- projection outputs ----
            qT = persist.tile([D, HALF], BF16, tag="qT")
            kT = persist.tile([D, S], BF16, tag="kT")
            vT = persist.tile([D, S], BF16, tag="vT")
            v_sb = persist.tile([128, NKC * D], BF16, tag="v")

            def project(col_group, dst, dst_off, sq, blk, bias_idx):
                """One 512-wide projection block: dst[:, dst_off:+512]."""
                ps = ps_main.tile([128, QB], F32, tag="ps_main")
                for e in range(NE):
                    nc.tensor.matmul(
                        ps[:],
                        lhsT=w_sb[
                            :, e * 3 * D + col_group * D : e * 3 * D + (col_group + 1) * D
                        ],
                        rhs=xt[(sq, e)][:, blk * QB : (blk + 1) * QB],
                        start=(e == 0),
                        stop=(e == NE - 1),
                    )
                nc.scalar.activation(
                    dst[:, dst_off : dst_off + QB],
                    ps[:],
                    AF.Identity,
                    bias=bias_sb[:, bias_idx : bias_idx + 1],
                )

            for sq in range(NQ):
                for blk in range(SQ // QB):
                    off = sq * SQ + blk * QB
                    if off < HALF:
                        project(0, qT, off, sq, blk, 0)
                    project(1, kT, off, sq, blk, 1)
                    project(2, vT, off, sq, blk, 2)
                # v chunks for this quarter: transpose vT[:, j*128:(j+1)*128]
                for j in range(sq * (SQ // 128), (sq + 1) * (SQ // 128)):
                    ps_t = ps_small.tile([128, 128], BF16, tag="ps_small")
                    nc.tensor.transpose(ps_t[:], vT[:, j * 128 : (j + 1) * 128], ident[:])
                    nc.vector.tensor_copy(v_sb[:, j * D : (j + 1) * D], ps_t[:])

            # ---- attention ----
            sums_sb = persist.tile([1, HALF], F32, tag="sums_sb")
            for qb in range(NQB):
                ps_o = ps_acc.tile([128, QB], F32, tag="ps_acc")
                ps_sum = ps_small.tile([1, QB], F32, tag="ps_small")
                for k in range(NKC):
                    ps_s = ps_main.tile([128, QB], F32, tag="ps_main")
                    nc.tensor.matmul(
                        ps_s[:],
                        lhsT=kT[:, k * 128 : (k + 1) * 128],
                        rhs=qT[:, qb * QB : (qb + 1) * QB],
                        start=True,
                        stop=True,
                    )
                    ex = exp_pool.tile([128, QB], BF16, tag="exp")
                    nc.scalar.activation(ex[:], ps_s[:], AF.Exp, scale=SCALE)
                    nc.tensor.matmul(
                        ps_o[:],
                        lhsT=v_sb[:, k * D : (k + 1) * D],
                        rhs=ex[:],
                        start=(k == 0),
                        stop=(k == NKC - 1),
                    )
                    nc.tensor.matmul(
                        ps_sum[:],
                        lhsT=ones_col[:],
                        rhs=ex[:],
                        start=(k == 0),
                        stop=(k == NKC - 1),
                    )
                o_sb = out_pool.tile([128, QB], F32, tag="osb")
                nc.vector.tensor_copy(o_sb[:], ps_o[:])
                nc.sync.dma_start(out_d[:, qb * QB : (qb + 1) * QB], o_sb[:])
                nc.vector.tensor_copy(sums_sb[:, qb * QB : (qb + 1) * QB], ps_sum[:])
            nc.sync.dma_start(sums_d[:], sums_sb[:])

    nc.compile()
    return nc


def _prep_inputs(x, W, b):
    """Host-side sharding prep: cast to bf16, roll each core's batch."""
    x_bf = np.asarray(x).astype(ml_dtypes.bfloat16)
    w_bf = np.ascontiguousarray(np.asarray(W).astype(ml_dtypes.bfloat16))
    b_f = np.asarray(b, dtype=np.float32)
    bias_cols = np.ascontiguousarray(b_f.reshape(3, D).T)  # [128, 3]
    in_maps = []
    for c in range(N_CORES):
        bb, h = c // 2, c % 2
        xb = x_bf[bb]
        if h:
            xc = np.ascontiguousarray(np.concatenate([xb[HALF:], xb[:HALF]], axis=0))
        else:
            xc = np.ascontiguousarray(xb)
        in_maps.append({"x": xc, "w": w_bf, "bias_cols": bias_cols})
    return in_maps


def _run(in_maps, trace=False, trace_kwargs=None):
    if "nc" not in _CACHE:
        _CACHE["nc"] = _build()
    return run_bass_kernel_spmd(
        _CACHE["nc"],
        in_maps,
        list(range(N_CORES)),
        trace=trace,
        **(trace_kwargs or {}),
    )


def kernel(x, W, b):
    in_maps = _prep_inputs(x, W, b)
    res = _run(in_maps)
    out = np.empty((B, S, D), dtype=np.float32)
    for c in range(N_CORES):
        bb, h = c // 2, c % 2
        o_t = res.results[c]["out_t"]  # [D, HALF]
        sums = res.results[c]["sums"]  # [1, HALF]
        out[bb, h * HALF : (h + 1) * HALF, :] = (o_t / sums).T
    return out


# revision 2
# speedup vs baseline: 1.3572x; 1.3572x over previous
"""Single-head attention (B=4, S=4096, E=2048, d=128) on 8 trn2 cores.

Sharding: core c handles (batch b = c//2, seq half h = c%2). Each core
computes the full K/V projection for its batch (redundantly within the
pair) and attention for its 2048-query half. Host prep ships x already
transposed (xT [E, S], bf16) and rolled along S so the core's query rows
are always xT columns 0:2048 — softmax over keys is permutation-
invariant, so rolling K/V order is harmless and keeps the SPMD program
identical across cores.

Per-core pipeline (matmuls bf16, fp32 PSUM accumulation):
  xT tiles  <- plain DMA loads [128e x 1024s] x 64
  qT/kT/vT  <- W-stationary projection, bias folded into the ACT
               PSUM->SBUF evacuation (Identity activation, bias AP)
  v         <- PE transpose of vT chunks
  per k-pair: scoresT[k, q] = kT_chunk^T @ qT  (2 matmuls into one
               [128 x 1024] PSUM tile), one Exp over both chunks
               (scale=1/sqrt(d) folded in; no max subtraction needed:
               scores ~ N(0,1), exp is fp32-safe), 2 PV matmuls
               accumulating out_T[d, q].
  softmax denominators: DVE pair-sums of exp tiles (depth SUM_TREE),
               then exact ones-column matmul accumulation in PSUM.
Host: out = (out_T / sums).T per core, reassembled into [4,4096,128].
"""

import numpy as np
import ml_dtypes

import concourse.bass as bass
import concourse.tile as tile
from concourse import bacc, mybir
from concourse.bass_utils import run_bass_kernel_spmd
from concourse.masks import make_identity

N_CORES = 8
B, S, E, D = 4, 4096, 2048, 128
HALF = S // 2  # queries per core
QB = 512  # query block (PSUM bank width in fp32)
SCALE = 1.0 / float(np.sqrt(D))
SUM_TREE = 1  # DVE pair-sum depth before the exact ones-matmul reduction

BF16 = mybir.dt.bfloat16
F32 = mybir.dt.float32
AF = mybir.ActivationFunctionType

_CACHE = {}


def _build():
    nc = bacc.Bacc(trn_type="TRN2", target_bir_lowering=False, debug=False)

    x_d = nc.dram_tensor("xt", [E, S], BF16, kind="ExternalInput").ap()
    w_d = nc.dram_tensor("w", [E, 3 * D], BF16, kind="ExternalInput").ap()
    bias_d = nc.dram_tensor("bias_cols", [D, 3], F32, kind="ExternalInput").ap()
    out_d = nc.dram_tensor("out_t", [D, HALF], F32, kind="ExternalOutput").ap()
    sums_d = nc.dram_tensor("sums", [1, HALF], F32, kind="ExternalOutput").ap()

    NE = E // 128  # 16 e-chunks
    NQ = 4  # s-quarters
    SQ = S // NQ  # 1024
    NKP = S // 256  # 16 k-pairs
    NQB = HALF // QB  # 4 query blocks

    with tile.TileContext(nc) as tc:
        with (
            tc.tile_pool(name="xt", bufs=40) as xt_pool,
            tc.tile_pool(name="wsb", bufs=1) as w_pool,
            tc.tile_pool(name="persist", bufs=1) as persist,
            tc.tile_pool(name="exp", bufs=4) as exp_pool,
            tc.tile_pool(name="comb", bufs=6) as comb_pool,
            tc.tile_pool(name="osb", bufs=3) as out_pool,
            tc.tile_pool(name="ps_big", bufs=2, space="PSUM") as ps_big,
            tc.tile_pool(name="ps_acc", bufs=2, space="PSUM") as ps_acc,
            tc.tile_pool(name="ps_small", bufs=2, space="PSUM") as ps_small,
        ):
            # ---- constants / small inputs (scalar HWDGE queue, parallel to x) ----
            w_sb = w_pool.tile([128, NE * 3 * D], BF16, tag="w")
            for e in range(NE):
                nc.scalar.dma_start(
                    w_sb[:, e * 3 * D : (e + 1) * 3 * D],
                    w_d[e * 128 : (e + 1) * 128, :],
                )
            bias_sb = persist.tile([D, 3], F32, tag="bias")
            nc.scalar.dma_start(bias_sb[:], bias_d[:])
            ones_col = persist.tile([128, 1], BF16, tag="ones")
            nc.gpsimd.memset(ones_col[:], 1.0)
            ident = persist.tile([128, 128], BF16, tag="ident")
            make_identity(nc, ident[:])

            # ---- x loads, per (quarter, e-chunk), in consumption order ----
            xt = {}
            for sq in range(NQ):
                for e in range(NE):
                    t = xt_pool.tile([128, SQ], BF16, tag="xt")
                    nc.sync.dma_start(
                        t[:], x_d[e * 128 : (e + 1) * 128, sq * SQ : (sq + 1) * SQ]
                    )
                    xt[(sq, e)] = t

            # ---- projection outputs ----
            qT = persist.tile([D, HALF], BF16, tag="qT")
            kT = persist.tile([D, S], BF16, tag="kT")
            vT = persist.tile([D, S], BF16, tag="vT")
            v_sb = persist.tile([128, S // 128 * D], BF16, tag="v")

            def project(col_group, dst, dst_off, sq, bias_idx):
                """One quarter-wide (1024) projection block."""
                ps = ps_big.tile([128, SQ], F32, tag="ps_big")
                for e in range(NE):
                    w_ap = w_sb[
                        :, e * 3 * D + col_group * D : e * 3 * D + (col_group + 1) * D
                    ]
                    for half in range(2):
                        nc.tensor.matmul(
                            ps[:, half * QB : (half + 1) * QB],
                            lhsT=w_ap,
                            rhs=xt[(sq, e)][:, half * QB : (half + 1) * QB],
                            start=(e == 0),
                            stop=(e == NE - 1),
                        )
                nc.scalar.activation(
                    dst[:, dst_off : dst_off + SQ],
                    ps[:],
                    AF.Identity,
                    bias=bias_sb[:, bias_idx : bias_idx + 1],
                )

            for sq in range(NQ):
                off = sq * SQ
                if off < HALF:
                    project(0, qT, off, sq, 0)
                project(1, kT, off, sq, 1)
                project(2, vT, off, sq, 2)
                # v chunks for this quarter: transpose vT[:, j*128:(j+1)*128]
                for j in range(sq * (SQ // 128), (sq + 1) * (SQ // 128)):
                    ps_t = ps_small.tile([128, 128], BF16, tag="ps_small")
                    nc.tensor.transpose(ps_t[:], vT[:, j * 128 : (j + 1) * 128], ident[:])
                    nc.vector.tensor_copy(v_sb[:, j * D : (j + 1) * D], ps_t[:])

            # ---- attention ----
            sums_sb = persist.tile([1, HALF], F32, tag="sums_sb")
            for qb in range(NQB):
                q_ap = qT[:, qb * QB : (qb + 1) * QB]
                ps_o = ps_acc.tile([128, QB], F32, tag="ps_acc")
                ps_sum = ps_small.tile([1, QB], F32, tag="ps_small")
                n_red = NKP >> (SUM_TREE - 1) if SUM_TREE else 2 * NKP
                red_i = 0
                level1 = []  # SUM_TREE>=2 staging
                for kp in range(NKP):
                    ps_s = ps_big.tile([128, 2 * QB], F32, tag="ps_big")
                    for half in range(2):
                        k = 2 * kp + half
                        nc.tensor.matmul(
                            ps_s[:, half * QB : (half + 1) * QB],
                            lhsT=kT[:, k * 128 : (k + 1) * 128],
                            rhs=q_ap,
                            start=True,
                            stop=True,
                        )
                    ex = exp_pool.tile([128, 2 * QB], BF16, tag="exp")
                    nc.scalar.activation(ex[:], ps_s[:], AF.Exp, scale=SCALE)
                    for half in range(2):
                        k = 2 * kp + half
                        nc.tensor.matmul(
                            ps_o[:],
                            lhsT=v_sb[:, k * D : (k + 1) * D],
                            rhs=ex[:, half * QB : (half + 1) * QB],
                            start=(k == 0),
                            stop=(k == 2 * NKP - 1),
                        )
                    # softmax denominator reduction
                    if SUM_TREE == 0:
                        for half in range(2):
                            nc.tensor.matmul(
                                ps_sum[:],
                                lhsT=ones_col[:],
                                rhs=ex[:, half * QB : (half + 1) * QB],
                                start=(red_i == 0),
                                stop=(red_i == n_red - 1),
                            )
                            red_i += 1
                        continue
                    comb = comb_pool.tile([128, QB], BF16, tag="comb")
                    nc.vector.tensor_add(comb[:], ex[:, 0:QB], ex[:, QB : 2 * QB])
                    if SUM_TREE == 1:
                        nc.tensor.matmul(
                            ps_sum[:],
                            lhsT=ones_col[:],
                            rhs=comb[:],
                            start=(red_i == 0),
                            stop=(red_i == n_red - 1),
                        )
                        red_i += 1
                    else:
                        level1.append(comb)
                        if len(level1) == 2:
                            comb2 = comb_pool.tile([128, QB], BF16, tag="comb")
                            nc.vector.tensor_add(comb2[:], level1[0][:], level1[1][:])
                            level1 = []
                            nc.tensor.matmul(
                                ps_sum[:],
                                lhsT=ones_col[:],
                                rhs=comb2[:],
                                start=(red_i == 0),
                                stop=(red_i == n_red - 1),
                            )
                            red_i += 1
                o_sb = out_pool.tile([128, QB], F32, tag="osb")
                nc.vector.tensor_copy(o_sb[:], ps_o[:])
                nc.sync.dma_start(out_d[:, qb * QB : (qb + 1) * QB], o_sb[:])
                nc.vector.tensor_copy(sums_sb[:, qb * QB : (qb + 1) * QB], ps_sum[:])
            nc.sync.dma_start(sums_d[:], sums_sb[:])

    nc.compile()
    return nc


def _prep_inputs(x, W, b):
    """Host-side sharding prep: cast bf16, transpose to xT, roll per half."""
    b_f = np.asarray(b, dtype=np.float32)
    bias_cols = np.ascontiguousarray(b_f.reshape(3, D).T)  # [128, 3]
    w_bf = np.ascontiguousarray(np.asarray(W).astype(ml_dtypes.bfloat16))
    in_maps = []
    for bb in range(B):
        xt_full = np.ascontiguousarray(
            np.asarray(x[bb]).astype(ml_dtypes.bfloat16).T
        )  # [E, S]
        for h in range(2):
            if h:
                xc = np.ascontiguousarray(
                    np.concatenate([xt_full[:, HALF:], xt_full[:, :HALF]], axis=1)
                )
            else:
                xc = xt_full
            in_maps.append({"xt": xc, "w": w_bf, "bias_cols": bias_cols})
    return in_maps


def _run(in_maps, trace=False, trace_kwargs=None):
    if "nc" not in _CACHE:
        _CACHE["nc"] = _build()
    return run_bass_kernel_spmd(
        _CACHE["nc"],
        in_maps,
        list(range(N_CORES)),
        trace=trace,
        **(trace_kwargs or {}),
    )


def kernel(x, W, b):
    in_maps = _prep_inputs(x, W, b)
    res = _run(in_maps)
    out = np.empty((B, S, D), dtype=np.float32)
    for c in range(N_CORES):
        bb, h = c // 2, c % 2
        o_t = res.results[c]["out_t"]  # [D, HALF]
        sums = res.results[c]["sums"]  # [1, HALF]
        out[bb, h * HALF : (h + 1) * HALF, :] = (o_t / sums).T
    return out
